# revision 1
# baseline (speedup 1.0000x reference)
"""GNN message-passing (2 hops, relu MLP mix) on 8 trn2 NeuronCores.

Strategy: shard nodes (and dst-grouped edges) across 8 cores. Per layer:
  - gather feats[src] rows (256B) from HBM via gpsimd dma_gather (edges land
    on partitions). dma_gather indices are int16, so the node table is split
    into two halves (A = cores 0-3, B = cores 4-7) with per-window A/B runs.
  - segment-sum by dst via TensorE: msgT = G.T @ S, where S = one-hot
    selector S[e, d] = (dst_local[e]==d) * w'[e], built with a single fused
    VectorE tensor_scalar (is_equal, mult) per 128-edge tile, accumulated in
    PSUM over each 128-dst window.
  - dense update via 3 PSUM-accumulating matmuls (feats@W0.T, msg@W1.T,
    rank-1 ones@bias) + relu on ScalarE.
  - inter-layer: AllGather collective distributes the updated shard to every
    core's local HBM (Shared HBM on trn2 is only pair-shared, so a plain
    shared buffer cannot implement the all-gather).
w' = w / (segment_sum(w)[dst] + eps) is folded in on the host, so no
normalization is needed on device.
"""

import sys

sys.path.insert(0, "/opt/trn_rl_repo")

from contextlib import ExitStack

import numpy as np

import concourse.bass as bass
import concourse.tile as tile
from concourse import bacc, library_config, mybir

N_NODES = 50000
D = 64
N_CORES = 8
NPC = N_NODES // N_CORES  # 6250 nodes per core
P = 128
NWIN = (NPC + P - 1) // P  # 49 windows of 128 dst nodes per core
PADN = NWIN * P  # 6272 padded rows per core in the all-gathered buffer
HALF1 = 4 * NPC  # layer-1 table split: nf[:25000] / nf[25000:]
HALF2 = 4 * PADN  # layer-2 split: f1_all[:25088] / f1_all[25088:]
EPS = 1e-9

f32 = mybir.dt.float32
i16 = mybir.dt.int16

_cache = {}


def _pack_idx(stream):
    """dma_gather index layout: idx i at [i%16 + 16k, i//16] for k in 0..7."""
    n = stream.shape[0]
    out = np.zeros((P, n // 16), np.int16)
    base = stream.reshape(n // 16, 16).T  # [16, n/16]
    for k in range(8):
        out[16 * k : 16 * (k + 1), :] = base
    return out


def _preprocess(node_feats, edge_src, edge_dst, edge_w):
    nf = np.ascontiguousarray(np.asarray(node_feats, np.float32))
    src = np.asarray(edge_src).astype(np.int64)
    dst = np.asarray(edge_dst).astype(np.int64)  # sorted by construction
    E = src.shape[0]

    denom = np.bincount(dst, weights=np.asarray(edge_w, np.float64), minlength=N_NODES)
    wp = (np.asarray(edge_w, np.float64) / (denom[dst] + EPS)).astype(np.float32)

    core = dst // NPC
    loc = dst % NPC
    win = loc // P
    dloc = (loc % P).astype(np.float32)
    is_b = (src >= HALF1).astype(np.int64)  # table half of the source node

    # order edges by (core, win, half, original); dst-sorted already gives
    # (core, win) grouping, so sort by half within each group (stable)
    order = np.lexsort((np.arange(E), is_b, win, core))
    src, dst, wp, core, win, dloc, is_b = (
        a[order] for a in (src, dst, wp, core, win, dloc, is_b)
    )

    gwin = core * NWIN + win
    key = gwin * 2 + is_b  # per (core, window, half) group
    counts = np.bincount(key, minlength=N_CORES * NWIN * 2)
    starts = np.concatenate([[0], np.cumsum(counts)[:-1]])
    pos = np.arange(E) - starts[key]

    ca = counts[0::2].reshape(N_CORES, NWIN)
    cb = counts[1::2].reshape(N_CORES, NWIN)
    TA = int(np.ceil(ca.max() / P))
    TB = int(np.ceil(cb.max() / P))
    GA, GB = NWIN * TA, NWIN * TB

    # stream position within each core's A (or B) stream
    T_of = np.where(is_b == 0, TA, TB)
    spos = (win * T_of * P + pos).astype(np.int64)

    idx1 = np.zeros((2, N_CORES, P, (GA * P) // 16), np.int16)
    idx2 = np.zeros((2, N_CORES, P, (GA * P) // 16), np.int16)
    dstloc = np.zeros((2, N_CORES, P, GA), np.float32)
    wparr = np.zeros((2, N_CORES, P, GA), np.float32)
    # half B arrays may have different G; allocate separately
    idx1b = np.zeros((N_CORES, P, (GB * P) // 16), np.int16)
    idx2b = np.zeros((N_CORES, P, (GB * P) // 16), np.int16)
    dstlocb = np.zeros((N_CORES, P, GB), np.float32)
    wparrb = np.zeros((N_CORES, P, GB), np.float32)

    r1 = np.where(is_b == 0, src, src - HALF1)  # layer-1 table row
    f1row = (src // NPC) * PADN + (src % NPC)
    r2 = np.where(is_b == 0, f1row, f1row - HALF2)  # layer-2 table row

    for k in range(N_CORES):
        for h in range(2):
            m = (core == k) & (is_b == h)
            G = GA if h == 0 else GB
            s1 = np.zeros(G * P, np.int64)
            s2 = np.zeros(G * P, np.int64)
            dl = np.zeros(G * P, np.float32)
            w_ = np.zeros(G * P, np.float32)
            sp = spos[m]
            s1[sp] = r1[m]
            s2[sp] = r2[m]
            dl[sp] = dloc[m]
            w_[sp] = wp[m]
            if h == 0:
                idx1[0, k] = _pack_idx(s1.astype(np.int16))
                idx2[0, k] = _pack_idx(s2.astype(np.int16))
                dstloc[0, k] = dl.reshape(G, P).T
                wparr[0, k] = w_.reshape(G, P).T
            else:
                idx1b[k] = _pack_idx(s1.astype(np.int16))
                idx2b[k] = _pack_idx(s2.astype(np.int16))
                dstlocb[k] = dl.reshape(G, P).T
                wparrb[k] = w_.reshape(G, P).T

    ft0 = np.zeros((N_CORES, D, PADN), np.float32)
    for k in range(N_CORES):
        ft0[k, :, :NPC] = nf[k * NPC : (k + 1) * NPC].T

    return dict(
        nf=nf,
        idx1a=idx1[0], idx2a=idx2[0], dla=dstloc[0], wpa=wparr[0],
        idx1b=idx1b, idx2b=idx2b, dlb=dstlocb, wpb=wparrb,
        ft0=ft0, TA=TA, TB=TB,
    )


def _build(TA, TB, variant="full"):
    """Build the SPMD Bacc program (identical for all 8 cores)."""
    GA, GB = NWIN * TA, NWIN * TB
    CH = 32  # gather chunk size in edge tiles (32*128 idxs = 1MB G buffer)

    nc = bacc.Bacc(num_swdge_queues=4)

    nf_d = nc.declare_dram_parameter("nf", [N_NODES, D], f32, isOutput=False)
    i1a_d = nc.declare_dram_parameter("idx1a", [P, GA * 8], i16, isOutput=False)
    i2a_d = nc.declare_dram_parameter("idx2a", [P, GA * 8], i16, isOutput=False)
    i1b_d = nc.declare_dram_parameter("idx1b", [P, GB * 8], i16, isOutput=False)
    i2b_d = nc.declare_dram_parameter("idx2b", [P, GB * 8], i16, isOutput=False)
    dla_d = nc.declare_dram_parameter("dla", [P, GA], f32, isOutput=False)
    wpa_d = nc.declare_dram_parameter("wpa", [P, GA], f32, isOutput=False)
    dlb_d = nc.declare_dram_parameter("dlb", [P, GB], f32, isOutput=False)
    wpb_d = nc.declare_dram_parameter("wpb", [P, GB], f32, isOutput=False)
    ft0_d = nc.declare_dram_parameter("ft0", [D, PADN], f32, isOutput=False)
    w0t_d = nc.declare_dram_parameter("w0t", [D, D], f32, isOutput=False)
    w1t_d = nc.declare_dram_parameter("w1t", [D, D], f32, isOutput=False)
    brow_d = nc.declare_dram_parameter("brow", [1, D], f32, isOutput=False)
    ones_d = nc.declare_dram_parameter("ones", [1, P], f32, isOutput=False)
    id_d = nc.declare_dram_parameter("ident", [P, P], f32, isOutput=False)
    iota_d = nc.declare_dram_parameter("iota", [P, P], f32, isOutput=False)
    out_d = nc.declare_dram_parameter("out", [NPC, D], f32, isOutput=True)

    f1_local = nc.dram_tensor("f1loc", [PADN, D], f32)
    f1_all = nc.dram_tensor("f1all", [N_CORES * PADN, D], f32, addr_space="Shared")

    with tile.TileContext(nc) as tc, ExitStack() as ctx:
        consts = ctx.enter_context(tc.tile_pool(name="consts", bufs=1))

        libload = nc.gpsimd.load_library(library_config.mlp)

        def load(dram, shape, dt):
            t = consts.tile(shape, dt, tag=dram.name + "_s")
            nc.sync.dma_start(t[:], dram[:])
            return t

        i1a_s = load(i1a_d, [P, GA * 8], i16)
        i2a_s = load(i2a_d, [P, GA * 8], i16)
        i1b_s = load(i1b_d, [P, GB * 8], i16)
        i2b_s = load(i2b_d, [P, GB * 8], i16)
        dla_s = load(dla_d, [P, GA], f32)
        wpa_s = load(wpa_d, [P, GA], f32)
        dlb_s = load(dlb_d, [P, GB], f32)
        wpb_s = load(wpb_d, [P, GB], f32)
        ftA = load(ft0_d, [D, PADN], f32)
        w0t_s = load(w0t_d, [D, D], f32)
        w1t_s = load(w1t_d, [D, D], f32)
        brow_s = load(brow_d, [1, D], f32)
        ones_s = load(ones_d, [1, P], f32)
        id_s = load(id_d, [P, P], f32)
        iota_s = load(iota_d, [P, P], f32)

        ftB = consts.tile([D, PADN], f32, tag="ftB")
        msgT = consts.tile([D, PADN], f32, tag="msgT")
        msgN = consts.tile([P, NWIN, D], f32, tag="msgN")
        nfb1 = consts.tile([P, NWIN, D], f32, tag="nfb1")
        nfb2 = consts.tile([P, NWIN, D], f32, tag="nfb2")

        gpool = ctx.enter_context(tc.tile_pool(name="g", bufs=6))
        spool = ctx.enter_context(tc.tile_pool(name="s", bufs=6))
        mpsum = ctx.enter_context(tc.tile_pool(name="mp", bufs=2, space="PSUM"))
        dpsum = ctx.enter_context(tc.tile_pool(name="dp", bufs=2, space="PSUM"))
        tpsum = ctx.enter_context(tc.tile_pool(name="tp", bufs=2, space="PSUM"))

        qrr = [0]

        def layer(tabA, tabB, iA, iB, ftX, nfb, build_ftB):
            gtiles = {}

            def chunk(half, c):
                if variant == "nogather":
                    half, c = 0, 0  # single gathered chunk reused everywhere
                k = (half, c)
                if k not in gtiles:
                    G = GA if half == 0 else GB
                    tab = tabA if half == 0 else tabB
                    idx = iA if half == 0 else iB
                    n = min(CH, G - c * CH) * P
                    t = gpool.tile([P, CH, D], f32, tag="g")
                    if True:
                        gi = nc.gpsimd.dma_gather(
                            out_ap=t[:, : n // P, :],
                            in_ap=tab,
                            idxs_ap=idx[:, c * CH * 8 : c * CH * 8 + n // 16],
                            num_idxs=n,
                            num_idxs_reg=n,
                            elem_size=D,
                            single_packet=False,
                            queue_num=qrr[0] % 4,
                        )
                        tile.add_dep_helper(gi.ins, libload.ins, reason="lib")
                        qrr[0] += 1
                    gtiles[k] = t
                return gtiles[k]

            # message accumulation: per 128-dst window, TA+TB edge-tile matmuls
            for w in range(NWIN):
                pm = mpsum.tile([P, D], f32, tag="mp")
                for half, T_, dl_s, wp_s in (
                    (0, TA, dla_s, wpa_s),
                    (1, TB, dlb_s, wpb_s),
                ):
                    for t in range(T_):
                        g = w * T_ + t
                        c, slot = divmod(g, CH)
                        gt = chunk(half, c)
                        first = w == 0 and t == 0 and half == 0
                        if variant != "nosbuild" or first:
                            st = spool.tile([P, P], f32, tag="s")
                            nc.vector.tensor_scalar(
                                st[:],
                                iota_s[:],
                                dl_s[:, g : g + 1],
                                wp_s[:, g : g + 1],
                                op0=mybir.AluOpType.is_equal,
                                op1=mybir.AluOpType.mult,
                            )
                            layer.st = st
                        st = layer.st
                        nc.tensor.matmul(
                            pm[:],
                            lhsT=st[:],
                            rhs=gt[:, slot, :],
                            start=(half == 0 and t == 0),
                            stop=(half == 1 and t == TB - 1),
                        )
                nc.scalar.copy(msgN[:, w, :], pm[:])
                ptm = tpsum.tile([D, P], f32, tag="tp")
                nc.tensor.transpose(ptm[:], msgN[:, w, :], id_s[:])
                nc.scalar.copy(msgT[:, w * P : (w + 1) * P], ptm[:])

            # dense update per 128-node tile
            for t in range(NWIN):
                pd = dpsum.tile([P, D], f32, tag="dp")
                nc.tensor.matmul(
                    pd[:], lhsT=ftX[:, t * P : (t + 1) * P], rhs=w0t_s[:],
                    start=True, stop=False,
                )
                nc.tensor.matmul(
                    pd[:], lhsT=msgT[:, t * P : (t + 1) * P], rhs=w1t_s[:],
                    start=False, stop=False,
                )
                nc.tensor.matmul(
                    pd[:], lhsT=ones_s[:], rhs=brow_s[:], start=False, stop=True
                )
                nc.scalar.activation(
                    nfb[:, t, :], pd[:], mybir.ActivationFunctionType.Relu
                )
                if build_ftB:
                    pt = tpsum.tile([D, P], f32, tag="tp")
                    nc.tensor.transpose(pt[:], nfb[:, t, :], id_s[:])
                    nc.scalar.copy(ftB[:, t * P : (t + 1) * P], pt[:])

        # ---------------- layer 1 ----------------
        layer(
            nf_d[0:HALF1, :], nf_d[HALF1:N_NODES, :], i1a_s, i1b_s,
            ftA, nfb1, build_ftB=True,
        )

        # all-gather the updated feats into every core's local HBM
        f1v = f1_local.rearrange("(t p) f -> p t f", p=P)
        nc.sync.dma_start(f1v, nfb1[:, :, :])
        if variant != "nocollective":
            nc.gpsimd.collective_compute(
                "AllGather",
                mybir.AluOpType.bypass,
                replica_groups=[list(range(N_CORES))],
                ins=[f1_local[:]],
                outs=[f1_all[:]],
            )

        # ---------------- layer 2 ----------------
        layer(
            f1_all[0:HALF2, :], f1_all[HALF2 : N_CORES * PADN, :], i2a_s, i2b_s,
            ftB, nfb2, build_ftB=False,
        )

        # final output (6250 = 48*128 + 106 rows)
        nfull = (NPC // P) * P
        of = out_d[0:nfull, :].rearrange("(t p) f -> p t f", p=P)
        nc.sync.dma_start(of, nfb2[:, : NPC // P, :])
        nc.sync.dma_start(out_d[nfull:NPC, :], nfb2[0 : NPC - nfull, NPC // P, :])

    nc.finalize()
    return nc


def _run(inputs, trace=False, trace_kwargs=None):
    from concourse.bass_utils import run_bass_kernel_spmd

    prep = _preprocess(
        inputs["node_feats"], inputs["edge_src"], inputs["edge_dst"], inputs["edge_w"]
    )
    key = (prep["TA"], prep["TB"])
    if key not in _cache:
        _cache[key] = _build(*key)
    nc = _cache[key]

    W0 = np.asarray(inputs["W0"], np.float32)
    W1 = np.asarray(inputs["W1"], np.float32)
    b0 = np.asarray(inputs["b0"], np.float32)
    b1 = np.asarray(inputs["b1"], np.float32)

    common = dict(
        nf=prep["nf"],
        w0t=np.ascontiguousarray(W0.T),
        w1t=np.ascontiguousarray(W1.T),
        brow=(b0 + b1)[None, :].astype(np.float32),
        ones=np.ones((1, P), np.float32),
        ident=np.eye(P, dtype=np.float32),
        iota=np.tile(np.arange(P, dtype=np.float32), (P, 1)),
    )
    in_maps = [
        dict(
            common,
            idx1a=prep["idx1a"][k], idx2a=prep["idx2a"][k],
            idx1b=prep["idx1b"][k], idx2b=prep["idx2b"][k],
            dla=prep["dla"][k], wpa=prep["wpa"][k],
            dlb=prep["dlb"][k], wpb=prep["wpb"][k],
            ft0=prep["ft0"][k],
        )
        for k in range(N_CORES)
    ]

    res = run_bass_kernel_spmd(
        nc,
        in_maps,
        core_ids=list(range(N_CORES)),
        trace=trace,
        **(trace_kwargs or {}),
    )
    out = np.concatenate([res.results[k]["out"] for k in range(N_CORES)], axis=0)
    return out.astype(np.float32), res


def kernel(**inputs):
    out, _ = _run(inputs, trace=False)
    return out



# revision 5
# speedup vs baseline: 12.0623x; 12.0623x over previous
"""GNN message-passing (2 hops, relu MLP mix) on 8 trn2 NeuronCores.

Strategy: shard nodes (and dst-grouped edges) across 8 cores. Per layer:
  - gather feats[src] rows from HBM via gpsimd dma_gather in bf16 (128B/row,
    halving gather DMA time vs f32). dma_gather indices are int16, so the
    node table is split into two halves (A = cores 0-3, B = cores 4-7) with
    per-window A/B runs.
  - segment-sum by dst via TensorE: msg = S.T @ G, where S = one-hot
    selector S[e, d] = (dst_local[e]==d) * w'[e], built bf16 with a single
    fused VectorE tensor_scalar (is_equal, mult) per 128-edge tile,
    accumulated in PSUM (f32) over each 128-dst window.
  - dense update via 3 PSUM-accumulating bf16 matmuls (feats@W0.T, msg@W1.T,
    rank-1 ones@bias) + relu on ScalarE.
  - inter-layer: AllGather collective (bf16 payload) distributes the updated
    shard to every core's local HBM.
w' = w / (segment_sum(w)[dst] + eps) is folded in on the host; f32 scalars
feed the is_equal/mult, everything else on-device is bf16 with f32 PSUM
accumulation (final output written f32).

_build(reps=N) chains N identical executions back-to-back (dep-fenced) so a
single dispatch measures N serialized runs; timing slope isolates the
per-execution device time from dispatch overhead.
"""

import sys

sys.path.insert(0, "/opt/trn_rl_repo")

from contextlib import ExitStack

import ml_dtypes
import numpy as np

import concourse.bass as bass
import concourse.tile as tile
from concourse import bacc, library_config, mybir

N_NODES = 50000
D = 64
N_CORES = 8
NPC = N_NODES // N_CORES  # 6250 nodes per core
P = 128
NWIN = (NPC + P - 1) // P  # 49 windows of 128 dst nodes per core
PADN = NWIN * P  # 6272 padded rows per core in the all-gathered buffer
HALF1 = 4 * NPC  # layer-1 table split: nf[:25000] / nf[25000:]
HALF2 = 4 * PADN  # layer-2 split: f1_all[:25088] / f1_all[25088:]
EPS = 1e-9
CH = 64  # gather chunk size in edge tiles (one chunk = 64*128 idxs)
RW = 128  # gathered row width in bf16 elems: 256B = dma_gather granularity

f32 = mybir.dt.float32
bf16 = mybir.dt.bfloat16
i16 = mybir.dt.int16
npbf16 = ml_dtypes.bfloat16

_cache = {}


def _pack_idx(stream):
    """dma_gather index layout: idx i at [i%16 + 16k, i//16] for k in 0..7."""
    n = stream.shape[0]
    out = np.zeros((P, n // 16), np.int16)
    base = stream.reshape(n // 16, 16).T  # [16, n/16]
    for k in range(8):
        out[16 * k : 16 * (k + 1), :] = base
    return out


def _preprocess(node_feats, edge_src, edge_dst, edge_w):
    nf = np.asarray(node_feats, np.float32).astype(npbf16)
    nf_pad = np.zeros((N_NODES, RW), npbf16)
    nf_pad[:, :D] = nf
    src = np.asarray(edge_src).astype(np.int64)
    dst = np.asarray(edge_dst).astype(np.int64)  # sorted by construction
    E = src.shape[0]

    denom = np.bincount(dst, weights=np.asarray(edge_w, np.float64), minlength=N_NODES)
    wp = (np.asarray(edge_w, np.float64) / (denom[dst] + EPS)).astype(np.float32)

    core = dst // NPC
    loc = dst % NPC
    win = loc // P
    dloc = (loc % P).astype(np.float32)
    is_b = (src >= HALF1).astype(np.int64)  # table half of the source node

    # order edges by (core, win, half, original); dst-sorted already gives
    # (core, win) grouping, so sort by half within each group (stable)
    order = np.lexsort((np.arange(E), is_b, win, core))
    src, dst, wp, core, win, dloc, is_b = (
        a[order] for a in (src, dst, wp, core, win, dloc, is_b)
    )

    gwin = core * NWIN + win
    key = gwin * 2 + is_b  # per (core, window, half) group
    counts = np.bincount(key, minlength=N_CORES * NWIN * 2)
    starts = np.concatenate([[0], np.cumsum(counts)[:-1]])
    pos = np.arange(E) - starts[key]

    ca = counts[0::2].reshape(N_CORES, NWIN)
    cb = counts[1::2].reshape(N_CORES, NWIN)
    TA = int(np.ceil(ca.max() / P))
    TB = int(np.ceil(cb.max() / P))
    GA, GB = NWIN * TA, NWIN * TB

    # stream position within each core's A (or B) stream
    T_of = np.where(is_b == 0, TA, TB)
    spos = (win * T_of * P + pos).astype(np.int64)

    idx1 = np.zeros((2, N_CORES, P, (GA * P) // 16), np.int16)
    idx2 = np.zeros((2, N_CORES, P, (GA * P) // 16), np.int16)
    dstloc = np.zeros((2, N_CORES, P, GA), np.float32)
    wparr = np.zeros((2, N_CORES, P, GA), np.float32)
    # half B arrays may have different G; allocate separately
    idx1b = np.zeros((N_CORES, P, (GB * P) // 16), np.int16)
    idx2b = np.zeros((N_CORES, P, (GB * P) // 16), np.int16)
    dstlocb = np.zeros((N_CORES, P, GB), np.float32)
    wparrb = np.zeros((N_CORES, P, GB), np.float32)

    r1 = np.where(is_b == 0, src, src - HALF1)  # layer-1 table row
    f1row = (src // NPC) * PADN + (src % NPC)
    r2 = np.where(is_b == 0, f1row, f1row - HALF2)  # layer-2 table row

    for k in range(N_CORES):
        for h in range(2):
            m = (core == k) & (is_b == h)
            G = GA if h == 0 else GB
            s1 = np.zeros(G * P, np.int64)
            s2 = np.zeros(G * P, np.int64)
            dl = np.zeros(G * P, np.float32)
            w_ = np.zeros(G * P, np.float32)
            sp = spos[m]
            s1[sp] = r1[m]
            s2[sp] = r2[m]
            dl[sp] = dloc[m]
            w_[sp] = wp[m]
            if h == 0:
                idx1[0, k] = _pack_idx(s1.astype(np.int16))
                idx2[0, k] = _pack_idx(s2.astype(np.int16))
                dstloc[0, k] = dl.reshape(G, P).T
                wparr[0, k] = w_.reshape(G, P).T
            else:
                idx1b[k] = _pack_idx(s1.astype(np.int16))
                idx2b[k] = _pack_idx(s2.astype(np.int16))
                dstlocb[k] = dl.reshape(G, P).T
                wparrb[k] = w_.reshape(G, P).T

    ft0 = np.zeros((N_CORES, D, PADN), npbf16)
    for k in range(N_CORES):
        ft0[k, :, :NPC] = nf[k * NPC : (k + 1) * NPC].T

    return dict(
        nf=nf_pad,
        idx1a=idx1[0], idx2a=idx2[0], dla=dstloc[0], wpa=wparr[0],
        idx1b=idx1b, idx2b=idx2b, dlb=dstlocb, wpb=wparrb,
        ft0=ft0, TA=TA, TB=TB,
    )


def make_in_maps(prep, inputs):
    W0 = np.asarray(inputs["W0"], np.float32)
    W1 = np.asarray(inputs["W1"], np.float32)
    b0 = np.asarray(inputs["b0"], np.float32)
    b1 = np.asarray(inputs["b1"], np.float32)
    common = dict(
        nf=prep["nf"],
        w0t=np.ascontiguousarray(W0.T).astype(npbf16),
        w1t=np.ascontiguousarray(W1.T).astype(npbf16),
        brow=(b0 + b1)[None, :].astype(npbf16),
        ones=np.ones((1, P), np.float32).astype(npbf16),
        ident=np.eye(P, dtype=np.float32).astype(npbf16),
        iota=np.tile(np.arange(P, dtype=np.float32), (P, 1)).astype(npbf16),
    )
    return [
        dict(
            common,
            idx1a=prep["idx1a"][k], idx2a=prep["idx2a"][k],
            idx1b=prep["idx1b"][k], idx2b=prep["idx2b"][k],
            dla=prep["dla"][k], wpa=prep["wpa"][k],
            dlb=prep["dlb"][k], wpb=prep["wpb"][k],
            ft0=prep["ft0"][k],
        )
        for k in range(N_CORES)
    ]


def _build(TA, TB, variant="full", reps=1):
    """Build the SPMD Bacc program (identical for all 8 cores)."""
    GA, GB = NWIN * TA, NWIN * TB

    nc = bacc.Bacc(num_swdge_queues=4)

    nf_d = nc.declare_dram_parameter("nf", [N_NODES, RW], bf16, isOutput=False)
    i1a_d = nc.declare_dram_parameter("idx1a", [P, GA * 8], i16, isOutput=False)
    i2a_d = nc.declare_dram_parameter("idx2a", [P, GA * 8], i16, isOutput=False)
    i1b_d = nc.declare_dram_parameter("idx1b", [P, GB * 8], i16, isOutput=False)
    i2b_d = nc.declare_dram_parameter("idx2b", [P, GB * 8], i16, isOutput=False)
    dla_d = nc.declare_dram_parameter("dla", [P, GA], f32, isOutput=False)
    wpa_d = nc.declare_dram_parameter("wpa", [P, GA], f32, isOutput=False)
    dlb_d = nc.declare_dram_parameter("dlb", [P, GB], f32, isOutput=False)
    wpb_d = nc.declare_dram_parameter("wpb", [P, GB], f32, isOutput=False)
    ft0_d = nc.declare_dram_parameter("ft0", [D, PADN], bf16, isOutput=False)
    w0t_d = nc.declare_dram_parameter("w0t", [D, D], bf16, isOutput=False)
    w1t_d = nc.declare_dram_parameter("w1t", [D, D], bf16, isOutput=False)
    brow_d = nc.declare_dram_parameter("brow", [1, D], bf16, isOutput=False)
    ones_d = nc.declare_dram_parameter("ones", [1, P], bf16, isOutput=False)
    id_d = nc.declare_dram_parameter("ident", [P, P], bf16, isOutput=False)
    iota_d = nc.declare_dram_parameter("iota", [P, P], bf16, isOutput=False)
    out_d = nc.declare_dram_parameter("out", [NPC, D], f32, isOutput=True)

    f1_local = nc.dram_tensor("f1loc", [PADN, RW], bf16)
    f1_all = nc.dram_tensor("f1all", [N_CORES * PADN, RW], bf16, addr_space="Shared")

    with tile.TileContext(nc) as tc, ExitStack() as ctx:
        consts = ctx.enter_context(tc.tile_pool(name="consts", bufs=1))

        libload = nc.gpsimd.load_library(library_config.mlp)

        def load(dram, shape, dt):
            t = consts.tile(shape, dt, tag=dram.name + "_s")
            nc.sync.dma_start(t[:], dram[:])
            return t

        i1a_s = load(i1a_d, [P, GA * 8], i16)
        i2a_s = load(i2a_d, [P, GA * 8], i16)
        i1b_s = load(i1b_d, [P, GB * 8], i16)
        i2b_s = load(i2b_d, [P, GB * 8], i16)
        dla_s = load(dla_d, [P, GA], f32)
        wpa_s = load(wpa_d, [P, GA], f32)
        dlb_s = load(dlb_d, [P, GB], f32)
        wpb_s = load(wpb_d, [P, GB], f32)
        ftA = load(ft0_d, [D, PADN], bf16)
        w0t_s = load(w0t_d, [D, D], bf16)
        w1t_s = load(w1t_d, [D, D], bf16)
        brow_s = load(brow_d, [1, D], bf16)
        ones_s = load(ones_d, [1, P], bf16)
        id_s = load(id_d, [P, P], bf16)
        iota_s = load(iota_d, [P, P], bf16)

        ftB = consts.tile([D, PADN], bf16, tag="ftB")
        msgT = consts.tile([D, PADN], bf16, tag="msgT")
        msgN = consts.tile([P, NWIN, D], bf16, tag="msgN")
        nfb1 = consts.tile([P, NWIN, D], bf16, tag="nfb1")
        nfb2 = consts.tile([P, NWIN, D], f32, tag="nfb2")

        gpool = ctx.enter_context(tc.tile_pool(name="g", bufs=4))
        spool = ctx.enter_context(tc.tile_pool(name="s", bufs=6))
        mpsum = ctx.enter_context(tc.tile_pool(name="mp", bufs=2, space="PSUM"))
        dpsum = ctx.enter_context(tc.tile_pool(name="dp", bufs=2, space="PSUM"))
        tpsum = ctx.enter_context(tc.tile_pool(name="tp", bufs=2, space="PSUM"))

        qrr = [0]
        fence = [None]  # last instruction of the previous rep (rep chaining)

        def layer(tabA, tabB, iA, iB, ftX, nfb, build_ftB):
            gtiles = {}
            first_gathers = []

            def chunk(half, c):
                if variant == "nogather":
                    half, c = 0, 0  # single gathered chunk reused everywhere
                k = (half, c)
                if k not in gtiles:
                    G = GA if half == 0 else GB
                    tab = tabA if half == 0 else tabB
                    idx = iA if half == 0 else iB
                    n = min(CH, G - c * CH) * P
                    t = gpool.tile([P, CH, RW], bf16, tag="g")
                    gi = nc.gpsimd.dma_gather(
                        out_ap=t[:, : n // P, :],
                        in_ap=tab,
                        idxs_ap=idx[:, c * CH * 8 : c * CH * 8 + n // 16],
                        num_idxs=n,
                        num_idxs_reg=n,
                        elem_size=RW,
                        single_packet=False,
                        queue_num=qrr[0] % 4,
                    )
                    tile.add_dep_helper(gi.ins, libload.ins, reason="lib")
                    if fence[0] is not None:
                        tile.add_dep_helper(gi.ins, fence[0], reason="rep-fence")
                    qrr[0] += 1
                    gtiles[k] = t
                return gtiles[k]

            # message accumulation: per 128-dst window, TA+TB edge-tile matmuls
            for w in range(NWIN):
                pm = mpsum.tile([P, D], f32, tag="mp")
                for half, T_, dl_s, wp_s in (
                    (0, TA, dla_s, wpa_s),
                    (1, TB, dlb_s, wpb_s),
                ):
                    for t in range(T_):
                        g = w * T_ + t
                        c, slot = divmod(g, CH)
                        gt = chunk(half, c)
                        first = w == 0 and t == 0 and half == 0
                        if variant != "nosbuild" or first:
                            st = spool.tile([P, P], bf16, tag="s")
                            nc.vector.tensor_scalar(
                                st[:],
                                iota_s[:],
                                dl_s[:, g : g + 1],
                                wp_s[:, g : g + 1],
                                op0=mybir.AluOpType.is_equal,
                                op1=mybir.AluOpType.mult,
                            )
                            layer.st = st
                        st = layer.st
                        nc.tensor.matmul(
                            pm[:],
                            lhsT=st[:],
                            rhs=gt[:, slot, 0:D],
                            start=(half == 0 and t == 0),
                            stop=(half == 1 and t == TB - 1),
                        )
                nc.scalar.copy(msgN[:, w, :], pm[:])
                ptm = tpsum.tile([D, P], bf16, tag="tp")
                nc.tensor.transpose(ptm[:], msgN[:, w, :], id_s[:])
                nc.scalar.copy(msgT[:, w * P : (w + 1) * P], ptm[:])

            # dense update per 128-node tile
            for t in range(NWIN):
                pd = dpsum.tile([P, D], f32, tag="dp")
                nc.tensor.matmul(
                    pd[:], lhsT=ftX[:, t * P : (t + 1) * P], rhs=w0t_s[:],
                    start=True, stop=False,
                )
                nc.tensor.matmul(
                    pd[:], lhsT=msgT[:, t * P : (t + 1) * P], rhs=w1t_s[:],
                    start=False, stop=False,
                )
                nc.tensor.matmul(
                    pd[:], lhsT=ones_s[:], rhs=brow_s[:], start=False, stop=True
                )
                nc.scalar.activation(
                    nfb[:, t, :], pd[:], mybir.ActivationFunctionType.Relu
                )
                if build_ftB:
                    pt = tpsum.tile([D, P], bf16, tag="tp")
                    nc.tensor.transpose(pt[:], nfb[:, t, :], id_s[:])
                    nc.scalar.copy(ftB[:, t * P : (t + 1) * P], pt[:])

        for rep in range(reps):
            # ---------------- layer 1 ----------------
            layer(
                nf_d[0:HALF1, :], nf_d[HALF1:N_NODES, :], i1a_s, i1b_s,
                ftA, nfb1, build_ftB=True,
            )

            # all-gather the updated feats into every core's local HBM
            f1v = f1_local.rearrange("(t p) f -> p t f", p=P)
            nc.sync.dma_start(f1v[:, :, 0:D], nfb1[:, :, :])
            if variant != "nocollective":
                nc.gpsimd.collective_compute(
                    "AllGather",
                    mybir.AluOpType.bypass,
                    replica_groups=[list(range(N_CORES))],
                    ins=[f1_local[:]],
                    outs=[f1_all[:]],
                )

            # ---------------- layer 2 ----------------
            layer(
                f1_all[0:HALF2, :], f1_all[HALF2 : N_CORES * PADN, :], i2a_s,
                i2b_s, ftB, nfb2, build_ftB=False,
            )

            # final output (6250 = 48*128 + 106 rows)
            nfull = (NPC // P) * P
            of = out_d[0:nfull, :].rearrange("(t p) f -> p t f", p=P)
            d1 = nc.sync.dma_start(of, nfb2[:, : NPC // P, :])
            d2 = nc.sync.dma_start(
                out_d[nfull:NPC, :], nfb2[0 : NPC - nfull, NPC // P, :]
            )
            fence[0] = d2.ins

    nc.finalize()
    return nc


def _run(inputs, trace=False, trace_kwargs=None):
    from concourse.bass_utils import run_bass_kernel_spmd

    prep = _preprocess(
        inputs["node_feats"], inputs["edge_src"], inputs["edge_dst"], inputs["edge_w"]
    )
    key = (prep["TA"], prep["TB"])
    if key not in _cache:
        _cache[key] = _build(*key)
    nc = _cache[key]

    in_maps = make_in_maps(prep, inputs)

    res = run_bass_kernel_spmd(
        nc,
        in_maps,
        core_ids=list(range(N_CORES)),
        trace=trace,
        **(trace_kwargs or {}),
    )
    out = np.concatenate([res.results[k]["out"] for k in range(N_CORES)], axis=0)
    return out.astype(np.float32), res


def kernel(**inputs):
    out, _ = _run(inputs, trace=False)
    return out
